# revision 42
# baseline (speedup 1.0000x reference)
"""Self-contained Trainium2 Bass kernel for nn_MixtureOfExperts_515396075673.

MoE: T=4096 tokens, D=1024, H=2048, E=8 experts, top-2, SwiGLU.

Strategy (expert-parallel, routed):
  - 8 NeuronCores, one expert per core; router replicated in fp32 on every
    core (top-2 selection gaps can be ~1e-5, so the router must be fp32).
    Wr columns are permuted per core so this core's expert is column 0 —
    mask extraction is a static slice, and the renormalized top-2 gate is
    sigmoid(2*l0 - m1 - m2) (no softmax / Exp table needed).
  - Tokens are processed in 3 scan-groups of (2, 10, 20) token tiles with
    fixed per-group compact capacities (128, 384, 768); positions within a
    group are partition-major (tri-matmul prefix across partitions +
    in-partition scan), local to the group, so groups are fully independent
    and the expert pipeline starts right after the tiny first group.
  - Per group: ONE dma_scatter_add writes each selected token's
    (id, gate) fp32 pair to its compact slot in a DRAM buffer (dropped
    tokens go to a dump row); the wrapped int16 position list is built via
    a DRAM bounce and replicated to all 8 gpsimd partition groups. The
    compacted ids come back as fp32, converted on-chip to a replicated
    int16 idx list, then ONE dma_gather(transpose=True) gathers +
    transposes the selected token rows from xbf into xgT [128, kd, cap].
  - Expert SwiGLU in bf16 per group; router groups 1-2 are interleaved
    into the expert instruction stream so the PE never idles (keeps the
    2.4 GHz p-state); weights stream in chunks ordered behind the router
    stream; outputs are staged in bf16 and written once per group.
  - Host: scatter-add the 8 compacted outputs into the full [T, D] output.
"""
import sys
sys.path.insert(0, "/opt/trn_rl_repo")

import numpy as np
import ml_dtypes
import concourse.bass as bass
import concourse.mybir as mybir
import concourse.tile as tile
from concourse import bacc
from concourse.bass_utils import run_bass_kernel_spmd

FP32 = mybir.dt.float32
F16 = mybir.dt.float16
BF16 = mybir.dt.bfloat16
I32 = mybir.dt.int32
I16 = mybir.dt.int16

T = 4096          # tokens
D = 1024          # model dim
H = 2048          # hidden
E = 8             # experts
P = 128           # partitions
BIG = 30000.0

GROUPS = ((2, 128), (10, 384), (20, 768))   # (token tiles, compact capacity)
C = sum(cap for _, cap in GROUPS)           # 1280
KD = D // P       # 8 k-chunks over model dim
KH = H // P       # 16 k-chunks over hidden dim
NTT = T // P      # 32 token tiles
ES = 64           # scatter destination row stride (fp32) = 256B
SILU_MODE = "act"
RQ_EVERY = 1   # "act": one Silu activation (HW); "sigmoid": CoreSim-safe


def build_moe_program(n_iters=1):
    nc = bacc.Bacc("TRN2", target_bir_lowering=False, debug=False, num_devices=8)

    # ---- DRAM I/O ----
    xTr = nc.dram_tensor("xTr", [NTT, P, KD, P], FP32, kind="ExternalInput").ap()
    xbf = nc.dram_tensor("xbf", [T + 1, D], BF16, kind="ExternalInput").ap()
    Wr = nc.dram_tensor("Wr", [D, E], FP32, kind="ExternalInput").ap()
    tri = nc.dram_tensor("tri", [P, P], FP32, kind="ExternalInput").ap()
    idh = nc.dram_tensor("idh", [P, NTT], F16, kind="ExternalInput").ap()
    idl = nc.dram_tensor("idl", [P, NTT], F16, kind="ExternalInput").ap()
    iop = nc.dram_tensor("iop", [P, C], F16, kind="ExternalInput").ap()
    w1 = nc.dram_tensor("w1", [D, H], BF16, kind="ExternalInput").ap()
    w3 = nc.dram_tensor("w3", [D, H], BF16, kind="ExternalInput").ap()
    w2 = nc.dram_tensor("w2", [H, D], BF16, kind="ExternalInput").ap()

    yT_out = nc.dram_tensor("yT_out", [D, C], BF16, kind="ExternalOutput").ap()
    idx_out = nc.dram_tensor("idx_out", [16, C // 16], I16,
                             kind="ExternalOutput").ap()

    with tile.TileContext(nc) as tc:
        def body():
            with (
                tc.tile_pool(name="const", bufs=1) as constp,
                tc.tile_pool(name="dram", bufs=1, space="DRAM") as dramp,
                tc.tile_pool(name="rt_sb", bufs=2) as rtp,
                tc.tile_pool(name="xr_sb", bufs=4) as xrp,
                tc.tile_pool(name="rt_ps", bufs=1, space="PSUM") as rtps,
                tc.tile_pool(name="ex_ps2", bufs=2, space="PSUM") as exps,
                tc.tile_pool(name="ex_sb", bufs=2) as exsb,
                tc.tile_pool(name="h_sb", bufs=2) as hp,
                tc.tile_pool(name="y_sb", bufs=1) as yp,
                tc.tile_pool(name="pay_sb", bufs=1) as payp,
            ):
                # ---- router weights first (tiny), then the first router
                # group's activation stream so the pipeline head is never
                # stuck behind bulk DMA
                Wr_sb = constp.tile([P, KD, E], FP32)
                nc.sync.dma_start(Wr_sb[:], Wr.rearrange("(k p) e -> p k e", p=P))
                xr_tiles = {}
                for tt in range(GROUPS[0][0]):
                    xr_tiles[tt] = xrp.tile([P, KD, P], FP32, tag="xr", name=f"xr{tt}")
                    nc.sync.dma_start(xr_tiles[tt][:], xTr[tt])
                tri_sb = constp.tile([P, P], FP32)
                nc.sync.dma_start(tri_sb[:], tri[:])
                idh_sb = constp.tile([P, NTT], F16)
                nc.sync.dma_start(idh_sb[:], idh[:])
                idl_sb = constp.tile([P, NTT], F16)
                nc.sync.dma_start(idl_sb[:], idl[:])
                iop_sb = constp.tile([P, C], F16)
                nc.sync.dma_start(iop_sb[:], iop[:])
                si16 = constp.tile([P, C // 16], I16)
                for mp in (32, 64, 96):
                    nc.vector.memset(si16[mp:mp + 32, :], 0)

                # resident weights + staging for gathered tokens
                w1sb = constp.tile([P, KD, H], BF16)
                w3sb = constp.tile([P, KD, H], BF16)
                w2sb = constp.tile([P, KH, D], BF16)
                xgT = [constp.tile([P, KD, cap], BF16, name=f"xgT{g}")
                       for g, (_, cap) in enumerate(GROUPS)]
                gate_row = constp.tile([1, C], FP32)
                gate_bc = constp.tile([P, C], FP32)

                # weight DMA chunks (mc-pairs), issued behind the router
                w1r = w1.rearrange("(k p) h -> p k h", p=P)
                w3r = w3.rearrange("(k p) h -> p k h", p=P)
                w2r = w2.rearrange("(k p) d -> p k d", p=P)
                def w13pair(k):
                    mc = 2 * k
                    return [(w1sb[:, :, mc * P:(mc + 2) * P],
                             w1r[:, :, mc * P:(mc + 2) * P]),
                            (w3sb[:, :, mc * P:(mc + 2) * P],
                             w3r[:, :, mc * P:(mc + 2) * P])]

                def w2pair(k):
                    dc = 2 * k
                    return [(w2sb[:, :, dc * P:(dc + 2) * P],
                             w2r[:, :, dc * P:(dc + 2) * P])]
                wq = []
                for k in range(4):
                    wq += w13pair(k)
                wq += w2pair(0) + w2pair(1)
                for k in range(4, 8):
                    wq += w13pair(k)
                wq += w2pair(2) + w2pair(3)
                wq.reverse()

                def issue_w(k):
                    for _ in range(min(k, len(wq))):
                        dst, src = wq.pop()
                        nc.sync.dma_start(dst, src)

                def fetch_xr(tt):
                    if tt not in xr_tiles and tt < NTT:
                        xr_tiles[tt] = xrp.tile([P, KD, P], FP32, tag="xr",
                                                name=f"xr{tt}")
                        nc.sync.dma_start(xr_tiles[tt][:], xTr[tt])

                def emit_router_tile(g, t, prefetch=0):
                    GT, cap = GROUPS[g]
                    t0 = sum(gt for gt, _ in GROUPS[:g])
                    tt = t0 + t
                    fetch_xr(tt)
                    for pf in range(1, prefetch + 1):
                        if t + pf < GT:
                            fetch_xr(tt + pf)
                    xr = xr_tiles[tt]
                    if t == 0:
                        rtps._g_psum = rtps.tile([P, GT * E + 1], FP32, tag="psl", name=f"psl{g}")
                    psum_l = rtps._g_psum
                    for kc in range(KD):
                        nc.tensor.matmul(
                            psum_l[:, t * E:(t + 1) * E],
                            lhsT=xr[:, kc, :], rhs=Wr_sb[:, kc, :],
                            start=(kc == 0), stop=(kc == KD - 1))
                    if t == GT - 1:
                        for q in range(t0, t0 + GT):
                            del xr_tiles[q]

                tail_state = {}

                def emit_router_tail_a(g):
                    GT, cap = GROUPS[g]
                    t0 = sum(gt for gt, _ in GROUPS[:g])
                    off = sum(c_ for _, c_ in GROUPS[:g])
                    psum_l = rtps._g_psum[:, 0:GT * E]
                    L3 = psum_l.rearrange("p (t e) -> p t e", e=E)
                    m1 = rtp.tile([P, GT], FP32, tag="m1")
                    nc.vector.reduce_max(m1[:, :, None], L3,
                                         axis=mybir.AxisListType.X)
                    eq = rtp.tile([P, GT * E], FP32, tag="eq")
                    eq3 = eq.rearrange("p (t e) -> p t e", e=E)
                    nc.vector.tensor_tensor(
                        out=eq3, in0=L3,
                        in1=m1[:, :, None].to_broadcast((P, GT, E)),
                        op=mybir.AluOpType.is_equal)
                    nc.vector.tensor_scalar_mul(eq[:], eq[:], -1e30)
                    lm3 = eq.rearrange("p (t e) -> p t e", e=E)
                    nc.vector.tensor_tensor(out=lm3, in0=lm3, in1=L3,
                                            op=mybir.AluOpType.add)
                    m2 = rtp.tile([P, GT], FP32, tag="m2")
                    nc.vector.reduce_max(m2[:, :, None], lm3,
                                         axis=mybir.AxisListType.X)
                    # this expert's logit is column 0 (host permuted Wr)
                    mask_g = rtp.tile([P, GT], FP32, tag="mask")
                    nc.vector.tensor_tensor(out=mask_g[:], in0=L3[:, :, 0],
                                            in1=m2[:], op=mybir.AluOpType.is_ge)
                    # renormalized top-2 gate: sigmoid(2*l0 - m1 - m2)
                    gg = rtp.tile([P, GT], FP32, tag="gg")
                    nc.vector.tensor_scalar_mul(gg[:], L3[:, :, 0], 2.0)
                    nc.vector.tensor_tensor(out=gg[:], in0=gg[:], in1=m1[:],
                                            op=mybir.AluOpType.subtract)
                    nc.vector.tensor_tensor(out=gg[:], in0=gg[:], in1=m2[:],
                                            op=mybir.AluOpType.subtract)
                    gate_g = rtp.tile([P, GT], FP32, tag="gate")
                    nc.scalar.activation(gate_g[:], gg[:],
                                         mybir.ActivationFunctionType.Sigmoid)

                    # ---- positions: partition-major compaction ----
                    incl = rtp.tile([P, GT], FP32, tag="incl")
                    nc.vector.tensor_tensor_scan(
                        out=incl[:], data0=mask_g[:], data1=mask_g[:],
                        initial=0.0, op0=mybir.AluOpType.add,
                        op1=mybir.AluOpType.bypass)
                    tot = rtp.tile([P, 1], FP32, tag="tot")
                    nc.vector.tensor_copy(tot[:], incl[:, GT - 1:GT])
                    tail_state[g] = (mask_g, gate_g, incl, tot)

                def emit_router_tail_b(g):
                    GT, cap = GROUPS[g]
                    t0 = sum(gt for gt, _ in GROUPS[:g])
                    off = sum(c_ for _, c_ in GROUPS[:g])
                    mask_g, gate_g, incl, tot = tail_state.pop(g)
                    ps_off = rtps._g_psum[:, GT * E:GT * E + 1]
                    nc.tensor.matmul(ps_off, lhsT=tri_sb[:], rhs=tot[:],
                                     start=True, stop=True)
                    pos = rtp.tile([P, GT], FP32, tag="pos")
                    nc.vector.tensor_tensor(out=pos[:], in0=incl[:],
                                            in1=mask_g[:],
                                            op=mybir.AluOpType.subtract)
                    nc.vector.tensor_scalar_add(pos[:], pos[:], ps_off)
                    # global compact position; non-selected pushed out of
                    # this group's iota range (dropped by the one-hot match)
                    pm = rtp.tile([P, GT], FP32, tag="pm")
                    nc.vector.tensor_scalar(
                        out=pm[:], in0=mask_g[:], scalar1=-BIG, scalar2=BIG,
                        op0=mybir.AluOpType.mult, op1=mybir.AluOpType.add)
                    nc.vector.tensor_tensor(out=pm[:], in0=pm[:], in1=pos[:],
                                            op=mybir.AluOpType.add)
                    nc.vector.tensor_scalar_add(pm[:], pm[:], float(off))

                    # payload (id_hi, id_lo, gate) in fp16, at stationary
                    # columns 0/32/64 so the psum rows land on partition
                    # starts the ISA can address (0, 32, 64)
                    pay = payp.tile([P, GT, 65], F16, tag="pay", name=f"pay{g}")
                    nc.vector.memset(pay[:], 0)
                    nc.vector.tensor_copy(pay[:, :, 0], idh_sb[:, t0:t0 + GT])
                    nc.vector.tensor_copy(pay[:, :, 32], idl_sb[:, t0:t0 + GT])
                    nc.vector.tensor_copy(pay[:, :, 64], gate_g[:])

                    # on-chip compaction: (hi, lo, gate) @ one_hot(pos).
                    # iop holds the wrap-PERMUTED slot index per column, so
                    # psum columns come out in dma_gather's wrapped idx
                    # order within each 16-partition x (cap/16) block.
                    nw = cap // 16
                    hsp = rtp.tile([16, nw], FP32, tag="hsp")
                    lsp = rtp.tile([16, nw], FP32, tag="lsp")
                    cw = (512 // nw) * nw
                    for o in range(0, cap, cw):
                        w = min(cw, cap - o)
                        cg = rtps.tile([65, w], FP32, tag="cg", name=f"cg{g}_{o}")
                        for t in range(GT):
                            oh = rtp.tile([P, w], F16, tag="oh")
                            nc.vector.tensor_scalar(
                                out=oh[:], in0=iop_sb[:, off + o:off + o + w],
                                scalar1=pm[:, t:t + 1], scalar2=None,
                                op0=mybir.AluOpType.is_equal)
                            nc.tensor.matmul(cg[:], lhsT=pay[:, t, :],
                                             rhs=oh[:],
                                             start=(t == 0), stop=(t == GT - 1))
                        # copy psum id rows to SBUF, then spread into the
                        # wrapped [16, nw] layout (DMA cannot read PSUM)
                        q0, q1 = o // nw, (o + w) // nw
                        hrow = rtp.tile([1, w], FP32, tag="hrow", name="hrow")
                        nc.vector.tensor_copy(hrow[:], cg[0:1, :])
                        lrow = rtp.tile([1, w], FP32, tag="lrow", name="lrow")
                        nc.vector.tensor_copy(lrow[:], cg[32:33, :])
                        nc.sync.dma_start(
                            hsp[q0:q1, :],
                            hrow.rearrange("o (q j) -> o q j", j=nw))
                        nc.sync.dma_start(
                            lsp[q0:q1, :],
                            lrow.rearrange("o (q j) -> o q j", j=nw))
                        # gate row to SBUF, then broadcast with columns
                        # un-permuted back to slot order (psum column
                        # q*nw+j holds slot j*16+q)
                        grow = rtp.tile([1, w], FP32, tag="grow", name="grow")
                        nc.vector.tensor_copy(grow[:], cg[64:65, :])
                        # gate_bc holds the PERMUTED row (column q*nw+j =
                        # slot j*16+q); ph2 un-permutes via a strided AP
                        nc.gpsimd.partition_broadcast(
                            gate_bc[:, off + o:off + o + w], grow[:])

                    # si16 = 256*hi + lo, replicated to partitions 16-31
                    wrp = rtp.tile([16, nw], FP32, tag="wrp")
                    nc.vector.tensor_scalar(
                        out=wrp[:], in0=hsp[:], scalar1=256.0, scalar2=None,
                        op0=mybir.AluOpType.mult)
                    nc.vector.tensor_tensor(out=wrp[:], in0=wrp[:], in1=lsp[:],
                                            op=mybir.AluOpType.add)
                    nc.vector.tensor_copy(
                        si16[0:16, off // 16:(off + cap) // 16], wrp[:])
                    nc.sync.dma_start(idx_out[:, off // 16:(off + cap) // 16],
                                      si16[0:16, off // 16:(off + cap) // 16])
                    nc.sync.dma_start(
                        si16[16:32, off // 16:(off + cap) // 16],
                        si16[0:16, off // 16:(off + cap) // 16])
                    nc.gpsimd.dma_gather(
                        out_ap=xgT[g][:], in_ap=xbf[:],
                        idxs_ap=si16[:, off // 16:(off + cap) // 16],
                        num_idxs=cap, num_idxs_reg=cap,
                        elem_size=D, transpose=True)

                # expert subtiles: (group, col offset within group, width)
                SUBS = []
                for g, (GT, cap) in enumerate(GROUPS):
                    o = 0
                    while o < cap:
                        w = min(512, cap - o)
                        SUBS.append((g, o, w))
                        o += w

                def emit_ph1(si_, mc):
                    g, o, w = SUBS[si_]
                    off = sum(c_ for _, c_ in GROUPS[:g])
                    if mc == 0:
                        hp._h = hp.tile([P, KH, w], BF16, tag="hT", name=f"hT{si_}")
                    hT = hp._h
                    ph1 = exps.tile([P, w], FP32, tag="ph1", name="ph1")
                    for kc in range(KD):
                        nc.tensor.matmul(
                            ph1[:], lhsT=w1sb[:, kc, mc * P:(mc + 1) * P],
                            rhs=xgT[g][:, kc, o:o + w],
                            start=(kc == 0), stop=(kc == KD - 1))
                    ph3 = exps.tile([P, w], FP32, tag="ph3", name="ph3")
                    for kc in range(KD):
                        nc.tensor.matmul(
                            ph3[:], lhsT=w3sb[:, kc, mc * P:(mc + 1) * P],
                            rhs=xgT[g][:, kc, o:o + w],
                            start=(kc == 0), stop=(kc == KD - 1))
                    sg = exsb.tile([P, w], BF16, tag="sg", name="sg")
                    if SILU_MODE == "act":
                        nc.scalar.activation(sg[:], ph1[:],
                                             mybir.ActivationFunctionType.Silu)
                        nc.vector.tensor_tensor(
                            out=hT[:, mc, :], in0=sg[:], in1=ph3[:],
                            op=mybir.AluOpType.mult)
                    else:
                        nc.scalar.activation(sg[:], ph1[:],
                                             mybir.ActivationFunctionType.Sigmoid)
                        nc.vector.tensor_tensor(
                            out=sg[:], in0=sg[:], in1=ph3[:],
                            op=mybir.AluOpType.mult)
                        nc.vector.tensor_tensor(
                            out=hT[:, mc, :], in0=sg[:], in1=ph1[:],
                            op=mybir.AluOpType.mult)

                def emit_ph2(si_, dc):
                    g, o, w = SUBS[si_]
                    off = sum(c_ for _, c_ in GROUPS[:g]) + o
                    hT = hp._h
                    if dc == 0:
                        yp._y = yp.tile([P, KD, w], BF16, tag="ysb", name=f"ysb{si_}")
                    ysb = yp._y
                    py = exps.tile([P, w], FP32, tag="py", name="py")
                    for hc in range(KH):
                        nc.tensor.matmul(
                            py[:], lhsT=w2sb[:, hc, dc * P:(dc + 1) * P],
                            rhs=hT[:, hc, :],
                            start=(hc == 0), stop=(hc == KH - 1))
                    goff = sum(c_ for _, c_ in GROUPS[:g])
                    nwg = GROUPS[g][1] // 16
                    j0 = o // 16
                    gap_ = (gate_bc[:, goff:goff + GROUPS[g][1]]
                            .rearrange("p (q j) -> p j q", j=nwg)
                            [:, j0:j0 + w // 16, :])
                    nc.vector.tensor_tensor(
                        out=ysb[:, dc, :].rearrange("p (j q) -> p j q", q=16),
                        in0=py.rearrange("p (j q) -> p j q", q=16),
                        in1=gap_, op=mybir.AluOpType.mult)
                    if dc == KD - 1:
                        nc.sync.dma_start(
                            yT_out[:, off:off + w]
                            .rearrange("(k p) c -> p k c", p=P), ysb[:])

                # ---------- emission schedule ----------
                # router g0 fully, then expert blocks with router g1/g2
                # tiles + weight chunks interleaved
                for t in range(GROUPS[0][0]):
                    emit_router_tile(0, t)
                emit_router_tail_a(0)
                emit_router_tail_b(0)

                rq = []
                for g in range(1, len(GROUPS)):
                    rq += ([(g, t) for t in range(GROUPS[g][0])]
                           + [(g, 'ta'), (g, 'tb')])

                def pop_rq():
                    g, t = rq.pop(0)
                    if t == 'ta':
                        emit_router_tail_a(g)
                    elif t == 'tb':
                        emit_router_tail_b(g)
                    else:
                        emit_router_tile(g, t, prefetch=2)

                # g1's router (and the start of g2's) runs while the g0
                # compaction chain resolves (the PE is DMA-paced here
                # anyway); the rest of g2's is paced into the expert stream
                PRE = GROUPS[1][0] + 2 + 8   # g1 tiles + tails + 8 g2 tiles
                for i in range(PRE):
                    if rq:
                        pop_rq()
                    issue_w(1)

                blocks = []
                for si_ in range(len(SUBS)):
                    blocks += [(si_, 'ph1', mc) for mc in range(KH)]
                    blocks += [(si_, 'ph2', dc) for dc in range(KD)]
                for bi, (si_, kind, i) in enumerate(blocks):
                    if kind == 'ph1':
                        emit_ph1(si_, i)
                    else:
                        emit_ph2(si_, i)
                    if rq and bi % RQ_EVERY == 0:
                        pop_rq()
                    issue_w(1)
                while rq:
                    pop_rq()

        if n_iters == 1:
            body()
        else:
            with tc.For_i(0, n_iters, 1):
                body()

    nc.compile()
    return nc


# ---------------- host side ----------------

def host_prepare(x, Wr, W1, W2, W3):
    """Build the 8 per-core input maps."""
    xf = np.ascontiguousarray(x.reshape(T, D).astype(np.float32))
    xTr_np = np.ascontiguousarray(
        xf.reshape(T // P, P, D // P, P).transpose(0, 3, 2, 1))
    xbf_np = np.zeros((T + 1, D), ml_dtypes.bfloat16)
    xbf_np[:T] = xf.astype(ml_dtypes.bfloat16)
    tri_np = np.triu(np.ones((P, P), np.float32), 1)
    tok = (np.arange(NTT)[None, :] * P + np.arange(P)[:, None])
    idh_np = (tok >> 8).astype(np.float16)
    idl_np = (tok & 255).astype(np.float16)
    # wrap-permuted iota: column off + q*(cap/16) + j holds global slot
    # off + j*16 + q, so the compaction matmul emits the idx list directly
    # in dma_gather's wrapped [16, cap/16] layout
    iop_np = np.zeros((P, C), np.float16)
    off = 0
    for gt, cap in GROUPS:
        nw = cap // 16
        cp = np.arange(cap)
        q, j = cp // nw, cp % nw
        iop_np[:, off:off + cap] = (off + j * 16 + q)[None, :].astype(
            np.float16)
        off += cap
    bf = ml_dtypes.bfloat16
    in_maps = []
    for c in range(E):
        perm = [c] + [e for e in range(E) if e != c]
        in_maps.append({
            "xTr": xTr_np, "xbf": xbf_np,
            "Wr": np.ascontiguousarray(Wr[:, perm].astype(np.float32)),
            "tri": tri_np, "idh": idh_np, "idl": idl_np, "iop": iop_np,
            "w1": np.ascontiguousarray(W1[c].astype(bf)),
            "w3": np.ascontiguousarray(W3[c].astype(bf)),
            "w2": np.ascontiguousarray(W2[c].astype(bf)),
        })
    return in_maps


def host_combine(results):
    out = np.zeros((T + 1, D), np.float32)
    for c in range(E):
        yT = results[c]["yT_out"].astype(np.float32)      # [D, C]
        wrapped = results[c]["idx_out"].astype(np.int64)  # [16, C//16]
        idx = np.zeros(C, np.int64)
        off = 0
        for gt, cap in GROUPS:
            nw = cap // 16
            blk = wrapped[:, off // 16:(off + cap) // 16]   # [16, nw]
            sl = np.arange(cap)
            idx[off + sl] = blk[sl % 16, sl // 16]
            off += cap
        np.add.at(out, idx, yT.T)
    return out[:T]


_PROGRAM_CACHE = {}


def kernel(x, Wr, W1, W2, W3):
    if "nc" not in _PROGRAM_CACHE:
        _PROGRAM_CACHE["nc"] = build_moe_program(1)
    nc = _PROGRAM_CACHE["nc"]
    in_maps = host_prepare(np.asarray(x), np.asarray(Wr), np.asarray(W1),
                           np.asarray(W2), np.asarray(W3))
    res = run_bass_kernel_spmd(nc, in_maps, list(range(E)))
    out = host_combine(res.results)
    return out.reshape(4, 1024, 1024).astype(np.float32)
